# revision 45
# baseline (speedup 1.0000x reference)
"""Trainium2 Bass kernel for BrainInspiredEmotionGraph (2-layer RGCN, 17 nodes,
8 relations, d=2048) running SPMD on 8 NeuronCores.

Math: layer(x) = sum_r A_r @ x @ W_r + x @ root + bias, where A_r is the
[17,17] per-relation mean-aggregation matrix built from the edge list.
h1 = relu(layer1(h)); out = layer2(h1), h = node_emb with signal rows patched.

Sharding (fully collective-free):
- Layer 1: output-column sharding. Core c computes h1[:, c*256:(c+1)*256]
  from W1[:, :, chunk] + root1[:, chunk] (host-premixed lhsT: (A_r h)^T per
  relation + h^T for the root, one long PSUM accumulation).
- Layer 2: hidden-dim contraction sharding. Core c computes the partial
  P_c = sum_r (A_r h1[:, chunk]) @ W2_r[chunk, :] + h1[:, chunk] @ root2[chunk, :]
  over the h1 columns it already owns — no inter-core exchange. The host
  sums the 8 [17, 2048] partials and adds bias2.

Precision/speed: every fp32 weight (and the layer-1 lhsT) is split on the
host into a bf16 (hi, lo) pair — identical HBM bytes, but each K-tile runs
as 3 bf16 matmuls (hi*hi + lo*hi + hi*lo, the lo*lo term is ~2^-16 and
dropped) at 1 cycle/row instead of fp32's 4, with fp32 PSUM accumulation.
Per-core HBM traffic is the roofline term: 37.75 MB streamed as contiguous
2 MB slabs (16 KB per partition per DMA).
"""
import sys

if '/opt/trn_rl_repo' not in sys.path:
    sys.path.insert(0, '/opt/trn_rl_repo')

import numpy as np
import ml_dtypes
from concourse import bacc, tile, mybir, bass_utils

BF16 = ml_dtypes.bfloat16
N_NODES = 17
N_REL = 8
D = 2048
N_CORES = 8
CH = D // N_CORES          # 256 columns of h1 owned per core
KT = 128                    # contraction rows per matmul
JT = D // KT                # 16 k-tiles per layer-1 slab
NSTRIP = 4                  # layer-2 output strips of 512 columns
F32 = mybir.dt.float32
BF = mybir.dt.bfloat16

NX = 9 * JT * N_NODES       # 2448 lhsT columns per hi/lo half
# fp32 const-tensor layout (word offsets): A_r^T stack, identity, b1, ones
OFF_AT = 0
OFF_ID = N_REL * N_NODES
OFF_B1 = 160
OFF_ONES = 416
CONSTF_W = 448

_compiled = None


def _build():
    nc = bacc.Bacc("TRN2", target_bir_lowering=False, debug=False,
                   num_devices=N_CORES)
    # layer-1 slabs: [128, 16 j-tiles * (hi,lo) * 256] bf16, K-permuted
    # (partition p holds rows {16p+j}); layer-2 slabs: [128, 2 kt * (hi,lo)
    # * 2048] bf16 (partition p holds rows p and 128+p of the 256-row band).
    w1 = nc.dram_tensor("w1", [9, KT, JT * 2 * CH], BF,
                        kind="ExternalInput").ap()
    w2 = nc.dram_tensor("w2", [9, KT, 4 * D], BF,
                        kind="ExternalInput").ap()
    xhl = nc.dram_tensor("xhl", [KT, 2 * NX], BF,
                         kind="ExternalInput").ap()
    cf = nc.dram_tensor("cf", [N_NODES, CONSTF_W], F32,
                        kind="ExternalInput").ap()
    out = nc.dram_tensor("out", [KT, NSTRIP * 512], F32,
                         kind="ExternalOutput").ap()

    with tile.TileContext(nc) as tc:
        with tc.tile_pool(name="const", bufs=1) as constp, \
             tc.tile_pool(name="wpool", bufs=8) as wpool, \
             tc.tile_pool(name="spool", bufs=2) as spool, \
             tc.tile_pool(name="opsum", bufs=1, space="PSUM") as opsum, \
             tc.tile_pool(name="ppsum", bufs=2, space="PSUM") as ppsum:

            xhl_sb = constp.tile([KT, 2 * NX], BF)
            # split so the layer-1 slab-0 lhsT tiles land first; cf (only
            # needed by the bias matmul, issued after slab 0) goes after
            nc.scalar.dma_start(out=xhl_sb[:, 0:JT * N_NODES],
                                in_=xhl[:, 0:JT * N_NODES])
            nc.scalar.dma_start(out=xhl_sb[:, NX:NX + JT * N_NODES],
                                in_=xhl[:, NX:NX + JT * N_NODES])
            cf_sb = constp.tile([N_NODES, CONSTF_W], F32)
            nc.scalar.dma_start(out=cf_sb, in_=cf)
            nc.scalar.dma_start(out=xhl_sb[:, JT * N_NODES:NX],
                                in_=xhl[:, JT * N_NODES:NX])
            nc.scalar.dma_start(out=xhl_sb[:, NX + JT * N_NODES:],
                                in_=xhl[:, NX + JT * N_NODES:])
            at_sb = cf_sb[:, OFF_AT:OFF_AT + N_REL * N_NODES]
            id_sb = cf_sb[:, OFF_ID:OFF_ID + N_NODES]
            b1_sb = cf_sb[0:1, OFF_B1:OFF_B1 + CH]
            ones_sb = cf_sb[0:1, OFF_ONES:OFF_ONES + N_NODES]

            def xh(k):
                return xhl_sb[:, k * N_NODES:(k + 1) * N_NODES]

            def xl(k):
                return xhl_sb[:, NX + k * N_NODES:NX + (k + 1) * N_NODES]

            # ---------------- layer 1 ----------------
            # col-tiled: M=17 uses 17 of 128 PE columns, so cycle matmuls
            # through 4 column groups (concurrent on HW); fold strips after.
            out1 = opsum.tile([KT, CH], F32, name="out1")
            started1 = [False] * 4
            mmi1 = [0]
            TOT1 = 1 + 9 * JT * 3

            def l1mm(lhsT, rhs):
                i = mmi1[0]
                g = i % 4
                mmi1[0] += 1
                nc.tensor.matmul(out1[32 * g:32 * g + N_NODES, :],
                                 lhsT=lhsT, rhs=rhs,
                                 start=not started1[g], stop=(i >= TOT1 - 4),
                                 tile_position=(0, 32 * g),
                                 skip_group_check=True)
                started1[g] = True

            for s in range(9):
                w = wpool.tile([KT, JT * 2 * CH], BF, name="wslab",
                               tag="wslab")
                if s == 0:
                    # stream the first slab in fine slices so PE starts early
                    # (few slices: each trigger costs ~0.6us of engine time)
                    cuts = [0, 512, 2048, 4096, JT * 2 * CH]
                    for a, b in zip(cuts[:-1], cuts[1:]):
                        nc.sync.dma_start(out=w[:, a:b], in_=w1[s][:, a:b])
                else:
                    nc.sync.dma_start(out=w, in_=w1[s])
                for j in range(JT):
                    k = s * JT + j
                    whi = w[:, (2 * j) * CH:(2 * j + 1) * CH]
                    wlo = w[:, (2 * j + 1) * CH:(2 * j + 2) * CH]
                    l1mm(xh(k), whi)
                    l1mm(xl(k), whi)
                    l1mm(xh(k), wlo)
                if s == 0:
                    # bias joins after slab 0 so PE start doesn't gate on cf
                    l1mm(ones_sb, b1_sb)
            # fold the 4 col-group strips (PSUM inputs may differ in base
            # partition; SB+SB may not)
            t0 = spool.tile([N_NODES, CH], F32, name="t0")
            t1 = spool.tile([N_NODES, CH], F32, name="t1")
            nc.vector.tensor_copy(t0, out1[0:N_NODES, :])
            nc.vector.tensor_add(t1, t0, out1[32:32 + N_NODES, :])
            nc.vector.tensor_add(t0, t1, out1[64:64 + N_NODES, :])
            s01 = spool.tile([N_NODES, CH], F32, name="s01")
            nc.vector.tensor_add(s01, t0, out1[96:96 + N_NODES, :])
            h1 = spool.tile([N_NODES, CH], F32, name="h1")
            nc.scalar.activation(h1, s01, mybir.ActivationFunctionType.Relu)

            # layer-2 lhsT prep: (A_r h1_c)^T for r<8 + h1_c^T for the root,
            # each split into bf16 hi/lo tiles
            xt2_hi = spool.tile([KT, 18 * N_NODES], BF, name="xt2_hi")
            xt2_lo = spool.tile([KT, 18 * N_NODES], BF, name="xt2_lo")
            for s in range(9):
                rhs = (at_sb[:, s * N_NODES:(s + 1) * N_NODES]
                       if s < N_REL else id_sb)
                for kt in range(2):
                    i = s * 2 + kt
                    sl = slice(i * N_NODES, (i + 1) * N_NODES)
                    pp = ppsum.tile([KT, N_NODES], F32, name="pp")
                    nc.tensor.matmul(pp, lhsT=h1[:, kt * KT:(kt + 1) * KT],
                                     rhs=rhs, start=True, stop=True)
                    nc.vector.tensor_copy(xt2_hi[:, sl], pp)
                    hi32 = spool.tile([KT, N_NODES], F32, name="hi32")
                    nc.vector.tensor_copy(hi32, xt2_hi[:, sl])
                    nc.vector.tensor_sub(xt2_lo[:, sl], pp, hi32)

            # ---------------- layer 2 (partial over owned h1 columns) -----
            out2 = []
            started2 = []
            mmi2 = []
            for n in range(NSTRIP):
                out2.append(opsum.tile([KT, 512], F32, name=f"out2_{n}",
                                       tag=f"out2_{n}"))
                started2.append([False] * 4)
                mmi2.append([0])
            TOT2 = 9 * 2 * 3

            def l2mm(n, lhsT, rhs):
                i = mmi2[n][0]
                g = (i + n) % 4  # offset by strip: no col-group collision
                mmi2[n][0] += 1
                nc.tensor.matmul(out2[n][32 * g:32 * g + N_NODES, :],
                                 lhsT=lhsT, rhs=rhs,
                                 start=not started2[n][g],
                                 stop=(i >= TOT2 - 4),
                                 tile_position=(0, 32 * g),
                                 skip_group_check=True)
                started2[n][g] = True

            # ship the raw [128, 512] col-group partials; host folds the 4
            # partition strips (avoids a ~12us serialized DVE/PE tail)
            osb = spool.tile([KT, NSTRIP * 512], F32, name="osb")

            def strip_out(pair):
                for n in pair:
                    nc.vector.tensor_copy(osb[:, n * 512:(n + 1) * 512],
                                          out2[n])
                a, b = pair[0] * 512, (pair[-1] + 1) * 512
                nc.scalar.dma_start(out=out[:, a:b], in_=osb[:, a:b])

            # root2 (slab 8) streams early into a dedicated buffer; slab 7
            # is processed last, quartered and strip-interleaved so the
            # output path overlaps the final arrivals.
            w8 = wpool.tile([KT, 4 * D], BF, name="w8", tag="w8", bufs=1)
            nc.sync.dma_start(out=w8, in_=w2[8])
            wtiles = {8: w8}
            for s in (0, 1, 2, 3, 4, 5, 6, 7):
                w = wpool.tile([KT, 4 * D], BF, name="wslab", tag="wslab")
                wtiles[s] = w
                if s == 7:
                    # eighths, ordered so strips (0,1) complete first
                    q8 = 4 * D // 8
                    for q in (0, 2, 4, 6, 1, 3, 5, 7):
                        nc.sync.dma_start(out=w[:, q * q8:(q + 1) * q8],
                                          in_=w2[s][:, q * q8:(q + 1) * q8])
                else:
                    nc.sync.dma_start(out=w, in_=w2[s])
            for s in (0, 1, 2, 3, 4, 5, 6, 8, 7):
                w = wtiles[s]
                strip_sets = ([(0, 1), (2, 3)] if s == 7
                              else [tuple(range(NSTRIP))])
                for strips in strip_sets:
                    for kt in range(2):
                        i = s * 2 + kt
                        lhi = xt2_hi[:, i * N_NODES:(i + 1) * N_NODES]
                        llo = xt2_lo[:, i * N_NODES:(i + 1) * N_NODES]
                        for n in strips:
                            whi = w[:, (2 * kt) * D + n * 512:
                                    (2 * kt) * D + (n + 1) * 512]
                            wlo = w[:, (2 * kt + 1) * D + n * 512:
                                    (2 * kt + 1) * D + (n + 1) * 512]
                            l2mm(n, lhi, whi)
                            l2mm(n, llo, whi)
                            l2mm(n, lhi, wlo)
                    if s == 7:
                        strip_out(strips)

    nc.compile()
    return nc


def _hilo(w):
    """Split fp32 array into bf16 (hi, lo)."""
    hi = w.astype(BF16)
    lo = (w - hi.astype(np.float32)).astype(BF16)
    return hi, lo


def _prep_inputs(inputs):
    """Host-side prep: A matrices, premixed layer-1 lhsT, per-core weights."""
    h = np.array(inputs['node_emb'], dtype=np.float32, copy=True)
    sf = np.asarray(inputs['signal_features'], dtype=np.float32)
    h[:sf.shape[0]] = sf
    src = np.asarray(inputs['edge_index'])[0].astype(np.int64)
    dst = np.asarray(inputs['edge_index'])[1].astype(np.int64)
    et = np.asarray(inputs['edge_type']).astype(np.int64)

    A = np.zeros((N_REL, N_NODES, N_NODES), np.float32)
    cnt = np.zeros((N_REL, N_NODES), np.float32)
    np.add.at(cnt, (et, dst), 1.0)
    np.add.at(A, (et, dst, src), 1.0)
    A /= np.maximum(cnt, 1.0)[:, :, None]

    # layer-1 lhsT: 9 slabs of (A_r h)^T (+ h^T for root), K-permuted so
    # partition p holds rows {16p+j}: [128, 2448] fp32 -> bf16 hi/lo halves
    Z = np.concatenate([np.einsum('rij,jd->rid', A, h).astype(np.float32),
                        h[None]], axis=0)           # [9,17,2048]
    x1t = (Z.transpose(0, 2, 1)
            .reshape(9, KT, JT, N_NODES)
            .transpose(1, 0, 2, 3)
            .reshape(KT, NX)).astype(np.float32)
    xhi, xlo = _hilo(x1t)
    xhl = np.concatenate([xhi, xlo], axis=1).copy()  # [128, 2*NX] bf16

    # A_r^T stacked along columns: at[n, r*17+m] = A[r][m, n]
    at = (A.transpose(0, 2, 1).transpose(1, 0, 2)
           .reshape(N_NODES, N_REL * N_NODES)).astype(np.float32)

    W1 = np.asarray(inputs['W1'], dtype=np.float32)
    W2 = np.asarray(inputs['W2'], dtype=np.float32)
    r1 = np.asarray(inputs['root1'], dtype=np.float32)
    r2 = np.asarray(inputs['root2'], dtype=np.float32)
    bias1 = np.asarray(inputs['bias1'], dtype=np.float32)
    W1full = np.concatenate([W1, r1[None]], axis=0)   # [9,2048,2048]
    W2full = np.concatenate([W2, r2[None]], axis=0)   # [9,2048,2048]

    cf = np.zeros((N_NODES, CONSTF_W), np.float32)
    cf[:, OFF_AT:OFF_AT + N_REL * N_NODES] = at
    cf[:, OFF_ID:OFF_ID + N_NODES] = np.eye(N_NODES)
    cf[0, OFF_ONES:OFF_ONES + N_NODES] = 1.0

    in_maps = []
    for c in range(N_CORES):
        cols = slice(c * CH, (c + 1) * CH)
        w1c = (W1full[:, :, cols]
               .reshape(9, KT, JT, CH))               # [9,128,16,256] f32
        h1c, l1c = _hilo(w1c)
        w1hl = (np.stack([h1c, l1c], axis=3)          # [9,128,16,2,256]
                .reshape(9, KT, JT * 2 * CH)).copy()
        w2c = (W2full[:, cols, :]
               .reshape(9, 2, KT, D)
               .transpose(0, 2, 1, 3))                # [9,128,2,2048] f32
        h2c, l2c = _hilo(w2c)
        w2hl = (np.stack([h2c, l2c], axis=3)          # [9,128,2,2,2048]
                .reshape(9, KT, 4 * D)).copy()
        cfc = cf.copy()
        cfc[0, OFF_B1:OFF_B1 + CH] = bias1[cols]
        in_maps.append({
            'w1': w1hl,
            'w2': w2hl,
            'xhl': xhl,
            'cf': cfc,
        })
    return in_maps


def get_compiled():
    global _compiled
    if _compiled is None:
        _compiled = _build()
    return _compiled


def run(inputs, trace=False):
    nc = get_compiled()
    in_maps = _prep_inputs(inputs)
    res = bass_utils.run_bass_kernel_spmd(
        nc, in_maps, core_ids=list(range(N_CORES)), trace=trace)
    acc = np.zeros((N_NODES, D), np.float64)
    for c in range(N_CORES):
        # out[32g+m, n*512+j] = col-group-g partial of P_c[m, n*512+j]
        o = np.asarray(res.results[c]['out'], dtype=np.float64)
        acc += o.reshape(4, 32, D)[:, :N_NODES, :].sum(axis=0)
    acc += np.asarray(inputs['bias2'], dtype=np.float64)[None, :]
    return acc.astype(np.float32), res


def kernel(**inputs):
    outp, _ = run(inputs, trace=False)
    return outp
